# revision 12
# baseline (speedup 1.0000x reference)
"""ColorizationNet Trainium2 kernel (8 NeuronCores, SPMD, two phases).

Structure exploited: rows of the big FC input [4096, 32786] share an identical
x_conv prefix (32768 cols), so

    fc_in @ w1.T = x_conv @ w1[:, :32768].T  (one shared matvec, [304])
                 + [pos|chunks] @ w1[:, 32768:].T  ([4096,18] GEMM)

Sharding (core r of 8):
  - conv backbone row-sharded: core r produces the x_conv slice for pooled
    rows [4r, 4r+4) of every channel (halos via zero-padded input windows,
    out-of-image "phantom" rows masked to zero via activation scale).
  - shared matvec K-sharded to match (each core streams 1/8 of w1's big
    part, 2.5 MB bf16, laid out so each SBUF partition's data is contiguous
    in DRAM).  Phase A outputs the 8 partials [304]; the host sums them
    (collectives are unavailable under the axon PJRT execution path).
  - phase B: patch FC sharded by patch row, core r handles patches
    [512r, 512(r+1)).

All matmul operands are bf16 (1 cycle/row on PE at any free size, vs 4 for
fp32); accumulation is fp32 in PSUM, and activation bias/scale constants
stay fp32.  Convs use a banded-rows formulation: moving operand = input
rows on SBUF partitions, stationary = banded weight matrix with output
columns m = (s, rowpair, c_out) so the 2x2 maxpool's vertical pair is
partitions p / p+64 (one tensor_max) and the horizontal pair is a stride-2
free-dim pair.  Each layer's pooled activation is written by ScalarE
directly into the next layer's moving-window tiles (no DRAM round trips).
"""

import sys

for _p in ("/opt/trn_rl_repo",):
    if _p not in sys.path:
        sys.path.insert(0, _p)

import numpy as np
import ml_dtypes
from contextlib import ExitStack

BF16 = ml_dtypes.bfloat16

IMG = 256
CS = 4
G = 64
H1 = 304
H2 = 176
OUT = 48
NCORES = 8

# phase-A packed bf16 consts (ONE DMA): [96, 1668]
#   s1 [18, 384] @0 (+ copy at rows 32..50), xs [50, 258] @1152,
#   xs2 [34, 258] @1410, s2 [80, 384] @384, s3 [96, 384] @768
# phase-A packed fp32 masks/biases: [96, 13]
#   mk1 [64,3] @0, bm1 [64,3] @3, mk2 [64,3] @6, bm2 [64,3] @9, bc3 [64,1] @12
CAW_W = 1152 + 258 + 258
XS_O = 1152
XS2_O = 1410
CAM_W = 13

# phase-B packed bf16 consts (ONE DMA): [128, 1440]
#   extrasT [18, 512] @0, w1eT [18, 304] @512, w2a/b [128, 176] @816/@992,
#   w2c [48, 176] @1168, w3a [128, 48] @1344, w3b [48, 48] @1392
# phase-B fp32 vector (ONE DMA, sent after phase A): [128, 6]
#   shc cols 0:3, b2a col 3, b2b col 4, b3 col 5
CBW_W = 1440
SHCB_W = 6


def _build_s1(c1_w):
    # [18, 3, 128]: rows i = in-row in window; cols m = s*64 + jp*8 + c
    s1 = np.zeros((18, 3, 128), np.float32)
    for dx in range(3):
        for s in range(2):
            for jp in range(8):
                j = 2 * jp + s
                for c in range(8):
                    m = s * 64 + jp * 8 + c
                    for dy in range(3):
                        s1[j + dy, dx, m] = c1_w[c, 0, dy, dx]
    return np.ascontiguousarray(s1.reshape(18, 3 * 128))


def _build_s2(c2_w):
    # [80, 3, 128]: rows k = delta*8 + ci (ci in 0..8); cols m = s*64+jp*16+co
    s2 = np.zeros((80, 3, 128), np.float32)
    for dx in range(3):
        for s in range(2):
            for jp in range(4):
                j2 = 2 * jp + s
                for co in range(16):
                    m = s * 64 + jp * 16 + co
                    for ci in range(8):
                        for dy in range(3):
                            s2[(j2 + dy) * 8 + ci, dx, m] = c2_w[co, ci, dy, dx]
    return np.ascontiguousarray(s2.reshape(80, 3 * 128))


def _build_s3(c3_w):
    # [96, 3, 128]: rows k = delta*16 + ci (ci in 0..16); cols m = s*64+jpp*32+co
    s3 = np.zeros((96, 3, 128), np.float32)
    for dx in range(3):
        for s in range(2):
            for jpp in range(2):
                j3 = 2 * jpp + s
                for co in range(32):
                    m = s * 64 + jpp * 32 + co
                    for ci in range(16):
                        for dy in range(3):
                            s3[(j3 + dy) * 16 + ci, dx, m] = c3_w[co, ci, dy, dx]
    return np.ascontiguousarray(s3.reshape(96, 3 * 128))


def _host_inputs(x, c1_w, c1_b, c2_w, c2_b, c3_w, c3_b, w1, b1, w2, b2, w3, b3):
    """Returns (in_maps_a, in_maps_b_partial, b1). Each phase-A map has
    'caw' [96, CAW_W] bf16 (weights + image windows), 'cam' [96, CAM_W] f32,
    'w1ps' [128, 9728] bf16.  Each phase-B map has 'cbw' [128, CBW_W] bf16;
    'shcb' [128, 6] f32 (shared vector + biases) is added after phase A."""
    x = np.asarray(x, np.float32).reshape(IMG, IMG)
    s1 = _build_s1(np.asarray(c1_w, np.float32))
    s2 = _build_s2(np.asarray(c2_w, np.float32))
    s3 = _build_s3(np.asarray(c3_w, np.float32))
    bc3 = np.tile(np.asarray(c3_b, np.float32), 2).reshape(64, 1)

    caw0 = np.zeros((96, CAW_W), np.float32)
    caw0[0:18, 0:384] = s1
    caw0[32:50, 0:384] = s1  # duplicate for the base-32 conv1 window
    caw0[0:80, 384:768] = s2
    caw0[0:96, 768:1152] = s3
    caw0 = caw0.astype(BF16)

    # phase-B packed consts (same for every core except extrasT)
    cbw0 = np.zeros((128, CBW_W), np.float32)
    w1eT = np.asarray(w1, np.float32)[:, 32768:].T  # [18, 304]
    w2T = np.asarray(w2, np.float32).T  # [304, 176]
    w3T = np.asarray(w3, np.float32).T  # [176, 48]
    cbw0[0:18, 512:816] = w1eT
    cbw0[0:128, 816:992] = w2T[0:128]
    cbw0[0:128, 992:1168] = w2T[128:256]
    cbw0[0:48, 1168:1344] = w2T[256:304]
    cbw0[0:128, 1344:1392] = w3T[0:128]
    cbw0[0:48, 1392:1440] = w3T[128:176]
    bias0 = np.zeros((128, 3), np.float32)
    bias0[0:128, 0] = np.asarray(b2, np.float32)[0:128]
    bias0[0:48, 1] = np.asarray(b2, np.float32)[128:176]
    bias0[0:48, 2] = np.asarray(b3, np.float32)

    w1bigT = np.ascontiguousarray(np.asarray(w1, np.float32)[:, :32768].T)  # [32768, 304]
    chunks = x.reshape(G, CS, G, CS).transpose(0, 2, 1, 3).reshape(G * G, CS * CS)
    pi = (np.arange(G * G) // G).astype(np.float32) * CS
    pj = (np.arange(G * G) % G).astype(np.float32) * CS

    P = np.arange(128)
    B = np.arange(32)
    c1b = np.asarray(c1_b, np.float32)
    c2b = np.asarray(c2_b, np.float32)

    maps_a, maps_b = [], []
    for r in range(NCORES):
        # xs: x rows [32r-7, 32r+43), cols padded by 1 each side
        xs = np.zeros((50, 258), np.float32)
        lo = 32 * r - 7
        hi = 32 * r + 43
        slo, shi = max(lo, 0), min(hi, IMG)
        xs[slo - lo : shi - lo, 1:257] = x[slo:shi, :]
        xs = xs.astype(BF16)
        caw = caw0.copy()
        caw[0:50, XS_O : XS_O + 258] = xs
        caw[0:34, XS2_O : XS2_O + 258] = xs[16:50]

        cam = np.zeros((96, CAM_W), np.float32)
        # row-validity masks (zero out-of-image "phantom" pooled rows)
        for b in range(3):
            for jp in range(8):
                valid = 0 <= (16 * r - 3 + 8 * b + jp) < 128
                cam[jp * 8 : jp * 8 + 8, 0 + b] = 1.0 if valid else 0.0
                cam[jp * 8 : jp * 8 + 8, 3 + b] = c1b if valid else 0.0
            for jp in range(4):
                valid = 0 <= (8 * r - 1 + 4 * b + jp) < 64
                cam[jp * 16 : jp * 16 + 16, 6 + b] = 1.0 if valid else 0.0
                cam[jp * 16 : jp * 16 + 16, 9 + b] = c2b if valid else 0.0
        cam[0:64, 12:13] = bc3

        # w1ps [128, 32*304]: w1ps[p, j*304+o] = w1[o, kglobal(p, j)],
        # kglobal = (p%32)*1024 + (4r + p//32)*32 + j
        kg = (P[None, :] % 32) * 1024 + (4 * r + P[None, :] // 32) * 32 + B[:, None]
        w1ps = np.ascontiguousarray(
            w1bigT[kg.ravel()].reshape(32, 128, 304).transpose(1, 0, 2).reshape(128, 32 * 304)
        ).astype(BF16)
        maps_a.append({"caw": caw, "cam": cam, "w1ps": w1ps})

        cbw = cbw0.copy()
        sl = slice(512 * r, 512 * (r + 1))
        cbw[0, 0:512] = pi[sl]
        cbw[1, 0:512] = pj[sl]
        cbw[2:18, 0:512] = chunks[sl].T
        maps_b.append({"cbw": cbw.astype(BF16)})
    return maps_a, maps_b, bias0, np.asarray(b1, np.float32)


def _mk_nc():
    import concourse.bacc as bacc

    # Bacc (not raw Bass): its compile() runs move_matmul_waits_to_ldweights /
    # generate_event_semaphores, required for the 1-wait-per-instruction
    # hardware constraint.
    return bacc.Bacc("TRN2", target_bir_lowering=False, debug=False, num_devices=NCORES)


def _build_phase_a():
    """Convs + sharded shared-matvec partial. Output: part [1, 304]."""
    import concourse.tile as tile
    from concourse import mybir

    f32 = mybir.dt.float32
    bf16 = mybir.dt.bfloat16
    AF = mybir.ActivationFunctionType
    nc = _mk_nc()

    def din(name, shape, dt=f32):
        return nc.dram_tensor(name, list(shape), dt, kind="ExternalInput").ap()

    caw_d = din("caw", (96, CAW_W), bf16)
    cam_d = din("cam", (96, CAM_W), f32)
    w1ps_d = din("w1ps", (128, 32 * 304), bf16)
    part_d = nc.dram_tensor("part", [1, 304], f32, kind="ExternalOutput").ap()

    with tile.TileContext(nc) as tc, ExitStack() as ctx:
        cpool = ctx.enter_context(tc.tile_pool(name="consts", bufs=1))
        spool = ctx.enter_context(tc.tile_pool(name="work", bufs=2))
        pconv = ctx.enter_context(tc.tile_pool(name="pconv", bufs=3, space="PSUM"))
        pmv = ctx.enter_context(tc.tile_pool(name="pmv", bufs=1, space="PSUM"))
        pwarm = ctx.enter_context(tc.tile_pool(name="pwarm", bufs=1, space="PSUM"))

        # warm the ScalarE activation-function table early (overlaps DMAs)
        scr = cpool.tile([1, 1], f32, tag="scr")
        nc.vector.memset(scr[:], 0.0)
        scr2 = cpool.tile([1, 1], f32, tag="scr2")
        nc.scalar.copy(scr2[:], scr[:])
        nc.scalar.activation(scr2[:], scr[:], AF.Relu)

        # PE warm-up spin: keep the PE busy during the DMA head so HAM
        # un-throttles (1.2 -> 2.4 GHz) before the real matmuls arrive.
        wz = cpool.tile([1, 512], bf16, tag="wz")
        nc.vector.memset(wz[:], 0.0)
        wps = pwarm.tile([1, 512], f32, tag="wps")
        for i in range(10):
            nc.tensor.matmul(wps[:], lhsT=wz[:, 0:1], rhs=wz[:], start=True, stop=True)

        # packed consts: ONE DMA each (per-DMA sequencer overhead is ~1.5us)
        cam_t = cpool.tile([96, CAM_W], f32, tag="cam")
        nc.scalar.dma_start(cam_t[:], cam_d)
        caw_t = cpool.tile([96, CAW_W], bf16, tag="caw")
        nc.sync.dma_start(caw_t[:], caw_d)

        # w1 stream: 4 chunk DMAs into one [128, 9728] tile, on the same SP
        # queue AFTER the control DMAs (queue FIFO keeps the small loads first)
        wst = cpool.tile([128, 32 * 304], bf16, tag="w1s")
        CH = 4
        chw = 32 * 304 // CH
        for c in range(CH):
            nc.sync.dma_start(wst[:, c * chw : (c + 1) * chw], w1ps_d[:, c * chw : (c + 1) * chw])

        def s1ap(dx, base):  # stationary for conv1, at partition base 0 or 32
            return caw_t[base : base + 18, 128 * dx : 128 * (dx + 1)]

        def s2ap(dx):
            return caw_t[0:80, 384 + 128 * dx : 384 + 128 * (dx + 1)]

        def s3ap(dx):
            return caw_t[0:96, 768 + 128 * dx : 768 + 128 * (dx + 1)]

        mk1 = lambda b, n=64: cam_t[0:n, 0 + b : 1 + b]
        bm1 = lambda b, n=64: cam_t[0:n, 3 + b : 4 + b]
        mk2 = lambda b, n=64: cam_t[0:n, 6 + b : 7 + b]
        bm2 = lambda b, n=64: cam_t[0:n, 9 + b : 10 + b]
        bc3 = cam_t[0:64, 12:13]

        # next-layer moving-window tiles (built in place by ScalarE writes)
        m2 = [cpool.tile([80, 130], bf16, tag=f"m2_{i}", name=f"m2_{i}") for i in range(3)]
        m3 = [cpool.tile([96, 66], bf16, tag=f"m3_{i}", name=f"m3_{i}") for i in range(2)]
        xc_t = cpool.tile([128, 32], bf16, tag="xc")
        for t in m2:
            nc.vector.memset(t[:], 0.0)
        for t in m3:
            nc.vector.memset(t[:], 0.0)

        def pool_to(ps, width):
            """psum [128, width] (m = (s, pair, c)) -> [64, width//2] max-pooled."""
            vtop = spool.tile([64, width], f32, tag=f"vt{width}")
            nc.scalar.copy(vtop[:], ps[0:64, :])
            v = spool.tile([64, width], f32, tag=f"v{width}")
            nc.vector.tensor_max(v[:], ps[64:128, :], vtop[:])
            vv = v[:].rearrange("p (x t) -> p x t", t=2)
            ph = spool.tile([64, width // 2], f32, tag=f"ph{width}")
            nc.vector.tensor_max(ph[:], vv[:, :, 0], vv[:, :, 1])
            return ph

        # ---- conv1: 3 blocks of 16 output rows -> M2 tiles
        # (row-offset into caw, col-offset of the image window, stationary base)
        win1 = [(0, XS_O, 0), (0, XS2_O, 0), (32, XS_O, 32)]
        for b in range(3):
            rlo, colo, base = win1[b]
            ps = pconv.tile([128, 256], f32, tag="cps")
            for dx in range(3):
                nc.tensor.matmul(
                    ps[:],
                    lhsT=s1ap(dx, base),
                    rhs=caw_t[rlo : rlo + 18, colo + dx : colo + dx + 256],
                    start=(dx == 0),
                    stop=(dx == 2),
                )
            ph = pool_to(ps, 256)  # [64, 128]: partition = jp*8+c, row = 8b+jp
            nc.scalar.activation(
                m2[b][0:64, 1:129], ph[:], AF.Relu, bias=bm1(b), scale=mk1(b)
            )
            if b >= 1:  # rows 8b, 8b+1 also tail rows 8..10 of previous window
                nc.scalar.activation(
                    m2[b - 1][64:80, 1:129],
                    ph[0:16, :],
                    AF.Relu,
                    bias=bm1(b, 16),
                    scale=mk1(b, 16),
                )

        # ---- conv2: 3 blocks of 8 output rows -> M3 tiles
        for b in range(3):
            ps = pconv.tile([128, 128], f32, tag="cps")
            for dx in range(3):
                nc.tensor.matmul(
                    ps[:],
                    lhsT=s2ap(dx),
                    rhs=m2[b][:, dx : dx + 128],
                    start=(dx == 0),
                    stop=(dx == 2),
                )
            ph = pool_to(ps, 128)  # [64, 64]: partition = jp'*16+co, row = 4b+jp'
            if b == 0:
                nc.scalar.activation(m3[0][0:64, 1:65], ph[:], AF.Relu, bias=bm2(0), scale=mk2(0))
            elif b == 1:
                nc.scalar.activation(m3[1][0:64, 1:65], ph[:], AF.Relu, bias=bm2(1), scale=mk2(1))
                nc.scalar.activation(
                    m3[0][64:96, 1:65], ph[0:32, :], AF.Relu, bias=bm2(1, 32), scale=mk2(1, 32)
                )
            else:
                nc.scalar.activation(
                    m3[1][64:96, 1:65], ph[0:32, :], AF.Relu, bias=bm2(2, 32), scale=mk2(2, 32)
                )

        # ---- conv3: 2 m-blocks of 4 output rows -> xc [128, 32]
        for g in range(2):
            ps = pconv.tile([128, 64], f32, tag="cps")
            for dx in range(3):
                nc.tensor.matmul(
                    ps[:],
                    lhsT=s3ap(dx),
                    rhs=m3[g][:, dx : dx + 64],
                    start=(dx == 0),
                    stop=(dx == 2),
                )
            ph = pool_to(ps, 64)  # [64, 32]
            nc.scalar.activation(xc_t[64 * g : 64 * g + 64, :], ph[:], AF.Relu, bias=bc3)

        # ---- shared matvec partial [1, 304]
        ps_mv = pmv.tile([1, 304], f32, tag="mv")
        for b in range(32):
            nc.tensor.matmul(
                ps_mv[:],
                lhsT=xc_t[:, b : b + 1],
                rhs=wst[:, 304 * b : 304 * (b + 1)],
                start=(b == 0),
                stop=(b == 31),
            )
        part_s = spool.tile([1, 304], f32, tag="part")
        nc.scalar.copy(part_s[:], ps_mv[:])
        nc.sync.dma_start(part_d, part_s[:])

    nc.compile()
    return nc


def _build_phase_b():
    """Patch FC for this core's 512 patches, given summed shared vector."""
    import concourse.tile as tile
    from concourse import mybir

    f32 = mybir.dt.float32
    bf16 = mybir.dt.bfloat16
    AF = mybir.ActivationFunctionType
    nc = _mk_nc()

    cbw_d = nc.dram_tensor("cbw", [128, CBW_W], bf16, kind="ExternalInput").ap()
    shcb_d = nc.dram_tensor("shcb", [128, SHCB_W], f32, kind="ExternalInput").ap()
    yout_d = nc.dram_tensor("yout", [48, 512], f32, kind="ExternalOutput").ap()

    mblk = [(0, 128), (128, 128), (256, 48)]
    qblk = [(0, 128), (128, 48)]

    with tile.TileContext(nc) as tc, ExitStack() as ctx:
        cpool = ctx.enter_context(tc.tile_pool(name="consts", bufs=1))
        fpool = ctx.enter_context(tc.tile_pool(name="fc", bufs=1))
        pfc = ctx.enter_context(tc.tile_pool(name="pfc", bufs=1, space="PSUM"))
        phh = ctx.enter_context(tc.tile_pool(name="phh", bufs=3, space="PSUM"))
        pwarm = ctx.enter_context(tc.tile_pool(name="pwarm", bufs=1, space="PSUM"))

        # warm the ScalarE activation-function table early (overlaps DMAs)
        scr = cpool.tile([1, 1], f32, tag="scr")
        nc.vector.memset(scr[:], 0.0)
        scr2 = cpool.tile([1, 1], f32, tag="scr2")
        nc.scalar.activation(scr2[:], scr[:], AF.Relu)
        nc.scalar.activation(scr2[:], scr[:], AF.Sigmoid)

        # PE warm-up spin during the DMA head (see phase A)
        wz = cpool.tile([1, 512], bf16, tag="wz")
        nc.vector.memset(wz[:], 0.0)
        wps = pwarm.tile([1, 512], f32, tag="wps")
        for i in range(10):
            nc.tensor.matmul(wps[:], lhsT=wz[:, 0:1], rhs=wz[:], start=True, stop=True)

        cbw = cpool.tile([128, CBW_W], bf16, tag="cbw")
        nc.sync.dma_start(cbw[:], cbw_d)
        shcb = cpool.tile([128, SHCB_W], f32, tag="shcb")
        nc.scalar.dma_start(shcb[:], shcb_d)

        extrasT = cbw[0:18, 0:512]
        w1eT = cbw[0:18, 512:816]
        w2T_t = [cbw[0:128, 816:992], cbw[0:128, 992:1168], cbw[0:48, 1168:1344]]
        w3T_t = [cbw[0:128, 1344:1392], cbw[0:48, 1392:1440]]
        b2c_t = [shcb[0:128, 3:4], shcb[0:48, 4:5]]
        b3c_t = shcb[0:48, 5:6]
        sh_t = [shcb[0:128, 0:1], shcb[0:128, 1:2], shcb[0:48, 2:3]]

        h1_t = []
        for i, (off, mb) in enumerate(mblk):
            ps_e = pfc.tile([mb, 512], f32, tag=f"pse{i}")
            nc.tensor.matmul(
                ps_e[:],
                lhsT=w1eT[:, off : off + mb],
                rhs=extrasT,
                start=True,
                stop=True,
            )
            h1 = fpool.tile([mb, 512], bf16, tag=f"h1{i}")
            from concourse import mybir as _mb
            nc.vector.tensor_scalar(h1[:], ps_e[:], sh_t[i], 0.0, _mb.AluOpType.add, _mb.AluOpType.max)
            h1_t.append(h1)

        h2_t = []
        for q, (qoff, mq) in enumerate(qblk):
            ps_h = phh.tile([mq, 512], f32, tag="psh")
            for i, (off, mb) in enumerate(mblk):
                nc.tensor.matmul(
                    ps_h[:],
                    lhsT=w2T_t[i][:, qoff : qoff + mq],
                    rhs=h1_t[i][:],
                    start=(i == 0),
                    stop=(i == 2),
                )
            h2 = fpool.tile([mq, 512], bf16, tag=f"h2{q}")
            nc.scalar.activation(h2[:], ps_h[:], AF.Relu, bias=b2c_t[q])
            h2_t.append(h2)

        ps_o = phh.tile([48, 512], f32, tag="psh")
        for q, (qoff, mq) in enumerate(qblk):
            nc.tensor.matmul(
                ps_o[:],
                lhsT=w3T_t[q],
                rhs=h2_t[q][:],
                start=(q == 0),
                stop=(q == 1),
            )
        outs = fpool.tile([48, 512], f32, tag="outs")
        nc.scalar.activation(outs[:], ps_o[:], AF.Sigmoid, bias=b3c_t)
        nc.sync.dma_start(yout_d, outs[:])

    nc.compile()
    return nc


def _shcb_pack(sh, bias0):
    shcb = np.zeros((128, SHCB_W), np.float32)
    shcb[0:128, 0] = sh[0:128]
    shcb[0:128, 1] = sh[128:256]
    shcb[0:48, 2] = sh[256:304]
    shcb[:, 3:6] = bias0
    return shcb


def _run(maps_a, maps_b, bias0, b1, trace=False, trace_cores=None):
    from concourse.bass_utils import run_bass_kernel_spmd

    nca = _build_phase_a()
    res_a = run_bass_kernel_spmd(
        nca, maps_a, list(range(NCORES)), trace=trace, trace_cores=trace_cores
    )
    sh = np.sum([res_a.results[r]["part"][0] for r in range(NCORES)], axis=0) + b1
    shcb = _shcb_pack(sh, bias0)
    for mb in maps_b:
        mb["shcb"] = shcb
    ncb = _build_phase_b()
    res_b = run_bass_kernel_spmd(
        ncb, maps_b, list(range(NCORES)), trace=trace, trace_cores=trace_cores
    )
    full = np.empty((G * G, OUT), np.float32)
    for r in range(NCORES):
        full[512 * r : 512 * (r + 1), :] = res_b.results[r]["yout"].T
    return full.reshape(3, IMG, IMG), res_a, res_b


def kernel(**inputs):
    maps_a, maps_b, bias0, b1 = _host_inputs(**inputs)
    out, _, _ = _run(maps_a, maps_b, bias0, b1)
    return out


if __name__ == "__main__":
    import reference

    inp = {k: np.asarray(v) for k, v in reference.setup_inputs().items()}
    got = kernel(**inp)
    exp = np.asarray(reference.reference(**reference.setup_inputs()))
    err = np.abs(got - exp).max() / max(np.abs(exp).max(), 1e-9)
    print("Relative error:", err)


# revision 14
# speedup vs baseline: 1.0545x; 1.0545x over previous
"""ColorizationNet Trainium2 kernel (8 NeuronCores, SPMD, two phases).

Structure exploited: rows of the big FC input [4096, 32786] share an identical
x_conv prefix (32768 cols), so

    fc_in @ w1.T = x_conv @ w1[:, :32768].T  (one shared matvec, [304])
                 + [pos|chunks] @ w1[:, 32768:].T  ([4096,18] GEMM)

Sharding (core r of 8):
  - conv backbone row-sharded: core r produces the x_conv slice for pooled
    rows [4r, 4r+4) of every channel (halos via zero-padded input windows,
    out-of-image "phantom" rows masked to zero via activation scale).
  - shared matvec K-sharded to match (each core streams 1/8 of w1's big
    part, 2.5 MB bf16, laid out so each SBUF partition's data is contiguous
    in DRAM).  Phase A outputs the 8 partials [304]; the host sums them
    (collectives are unavailable under the axon PJRT execution path).
  - phase B: patch FC sharded by patch row, core r handles patches
    [512r, 512(r+1)).

All matmul operands are bf16 (1 cycle/row on PE at any free size, vs 4 for
fp32); accumulation is fp32 in PSUM, and activation bias/scale constants
stay fp32.  Convs use a banded-rows formulation: moving operand = input
rows on SBUF partitions, stationary = banded weight matrix with output
columns m = (s, rowpair, c_out) so the 2x2 maxpool's vertical pair is
partitions p / p+64 (one tensor_max) and the horizontal pair is a stride-2
free-dim pair.  Each layer's pooled activation is written by ScalarE
directly into the next layer's moving-window tiles (no DRAM round trips).
"""

import sys

for _p in ("/opt/trn_rl_repo",):
    if _p not in sys.path:
        sys.path.insert(0, _p)

import numpy as np
import ml_dtypes
from contextlib import ExitStack

BF16 = ml_dtypes.bfloat16

IMG = 256
CS = 4
G = 64
H1 = 304
H2 = 176
OUT = 48
NCORES = 8

# phase-A packed bf16 consts (ONE DMA): [96, 1668]
#   s1 [18, 384] @0 (+ copy at rows 32..50), xs [50, 258] @1152,
#   xs2 [34, 258] @1410, s2 [80, 384] @384, s3 [96, 384] @768
# phase-A packed fp32 masks/biases: [96, 13]
#   mk1 [64,3] @0, bm1 [64,3] @3, mk2 [64,3] @6, bm2 [64,3] @9, bc3 [64,1] @12
CAW_W = 1152 + 258 + 258
XS_O = 1152
XS2_O = 1410
CAM_W = 13

# phase-B packed bf16 consts (ONE DMA): [128, 1440]
#   extrasT [18, 512] @0, w1eT [18, 304] @512, w2a/b [128, 176] @816/@992,
#   w2c [48, 176] @1168, w3a [128, 48] @1344, w3b [48, 48] @1392
# phase-B fp32 vector (ONE DMA, sent after phase A): [128, 6]
#   shc cols 0:3, b2a col 3, b2b col 4, b3 col 5
CBW_W = 1440
SHCB_W = 6


def _build_s1(c1_w):
    # [18, 3, 128]: rows i = in-row in window; cols m = s*64 + jp*8 + c
    s1 = np.zeros((18, 3, 128), np.float32)
    for dx in range(3):
        for s in range(2):
            for jp in range(8):
                j = 2 * jp + s
                for c in range(8):
                    m = s * 64 + jp * 8 + c
                    for dy in range(3):
                        s1[j + dy, dx, m] = c1_w[c, 0, dy, dx]
    return np.ascontiguousarray(s1.reshape(18, 3 * 128))


def _build_s2(c2_w):
    # [80, 3, 128]: rows k = delta*8 + ci (ci in 0..8); cols m = s*64+jp*16+co
    s2 = np.zeros((80, 3, 128), np.float32)
    for dx in range(3):
        for s in range(2):
            for jp in range(4):
                j2 = 2 * jp + s
                for co in range(16):
                    m = s * 64 + jp * 16 + co
                    for ci in range(8):
                        for dy in range(3):
                            s2[(j2 + dy) * 8 + ci, dx, m] = c2_w[co, ci, dy, dx]
    return np.ascontiguousarray(s2.reshape(80, 3 * 128))


def _build_s3(c3_w):
    # [96, 3, 128]: rows k = delta*16 + ci (ci in 0..16); cols m = s*64+jpp*32+co
    s3 = np.zeros((96, 3, 128), np.float32)
    for dx in range(3):
        for s in range(2):
            for jpp in range(2):
                j3 = 2 * jpp + s
                for co in range(32):
                    m = s * 64 + jpp * 32 + co
                    for ci in range(16):
                        for dy in range(3):
                            s3[(j3 + dy) * 16 + ci, dx, m] = c3_w[co, ci, dy, dx]
    return np.ascontiguousarray(s3.reshape(96, 3 * 128))


def _host_inputs(x, c1_w, c1_b, c2_w, c2_b, c3_w, c3_b, w1, b1, w2, b2, w3, b3):
    """Returns (in_maps_a, in_maps_b_partial, b1). Each phase-A map has
    'caw' [96, CAW_W] bf16 (weights + image windows), 'cam' [96, CAM_W] f32,
    'w1ps' [128, 9728] bf16.  Each phase-B map has 'cbw' [128, CBW_W] bf16;
    'shcb' [128, 6] f32 (shared vector + biases) is added after phase A."""
    x = np.asarray(x, np.float32).reshape(IMG, IMG)
    s1 = _build_s1(np.asarray(c1_w, np.float32))
    s2 = _build_s2(np.asarray(c2_w, np.float32))
    s3 = _build_s3(np.asarray(c3_w, np.float32))
    bc3 = np.tile(np.asarray(c3_b, np.float32), 2).reshape(64, 1)

    caw0 = np.zeros((96, CAW_W), np.float32)
    caw0[0:18, 0:384] = s1
    caw0[32:50, 0:384] = s1  # duplicate for the base-32 conv1 window
    caw0[0:80, 384:768] = s2
    caw0[0:96, 768:1152] = s3
    caw0 = caw0.astype(BF16)

    # phase-B packed consts (same for every core except extrasT)
    cbw0 = np.zeros((128, CBW_W), np.float32)
    w1eT = np.asarray(w1, np.float32)[:, 32768:].T  # [18, 304]
    w2T = np.asarray(w2, np.float32).T  # [304, 176]
    w3T = np.asarray(w3, np.float32).T  # [176, 48]
    cbw0[0:18, 512:816] = w1eT
    cbw0[0:128, 816:992] = w2T[0:128]
    cbw0[0:128, 992:1168] = w2T[128:256]
    cbw0[0:48, 1168:1344] = w2T[256:304]
    cbw0[0:128, 1344:1392] = w3T[0:128]
    cbw0[0:48, 1392:1440] = w3T[128:176]
    bias0 = np.zeros((128, 3), np.float32)
    bias0[0:128, 0] = np.asarray(b2, np.float32)[0:128]
    bias0[0:48, 1] = np.asarray(b2, np.float32)[128:176]
    bias0[0:48, 2] = np.asarray(b3, np.float32)

    w1bigT = np.ascontiguousarray(np.asarray(w1, np.float32)[:, :32768].T)  # [32768, 304]
    chunks = x.reshape(G, CS, G, CS).transpose(0, 2, 1, 3).reshape(G * G, CS * CS)
    pi = (np.arange(G * G) // G).astype(np.float32) * CS
    pj = (np.arange(G * G) % G).astype(np.float32) * CS

    P = np.arange(128)
    B = np.arange(32)
    c1b = np.asarray(c1_b, np.float32)
    c2b = np.asarray(c2_b, np.float32)

    maps_a, maps_b = [], []
    for r in range(NCORES):
        # xs: x rows [32r-7, 32r+43), cols padded by 1 each side
        xs = np.zeros((50, 258), np.float32)
        lo = 32 * r - 7
        hi = 32 * r + 43
        slo, shi = max(lo, 0), min(hi, IMG)
        xs[slo - lo : shi - lo, 1:257] = x[slo:shi, :]
        xs = xs.astype(BF16)
        caw = caw0.copy()
        caw[0:50, XS_O : XS_O + 258] = xs
        caw[0:34, XS2_O : XS2_O + 258] = xs[16:50]

        cam = np.zeros((96, CAM_W), np.float32)
        # row-validity masks (zero out-of-image "phantom" pooled rows)
        for b in range(3):
            for jp in range(8):
                valid = 0 <= (16 * r - 3 + 8 * b + jp) < 128
                cam[jp * 8 : jp * 8 + 8, 0 + b] = 1.0 if valid else 0.0
                cam[jp * 8 : jp * 8 + 8, 3 + b] = c1b if valid else 0.0
            for jp in range(4):
                valid = 0 <= (8 * r - 1 + 4 * b + jp) < 64
                cam[jp * 16 : jp * 16 + 16, 6 + b] = 1.0 if valid else 0.0
                cam[jp * 16 : jp * 16 + 16, 9 + b] = c2b if valid else 0.0
        cam[0:64, 12:13] = bc3

        # w1ps [128, 32*304]: w1ps[p, j*304+o] = w1[o, kglobal(p, j)],
        # kglobal = (p%32)*1024 + (4r + p//32)*32 + j
        kg = (P[None, :] % 32) * 1024 + (4 * r + P[None, :] // 32) * 32 + B[:, None]
        w1ps = np.ascontiguousarray(
            w1bigT[kg.ravel()].reshape(32, 128, 304).transpose(1, 0, 2).reshape(128, 32 * 304)
        ).astype(BF16)
        maps_a.append({"caw": caw, "cam": cam, "w1ps": w1ps})

        cbw = cbw0.copy()
        sl = slice(512 * r, 512 * (r + 1))
        cbw[0, 0:512] = pi[sl]
        cbw[1, 0:512] = pj[sl]
        cbw[2:18, 0:512] = chunks[sl].T
        maps_b.append({"cbw": cbw.astype(BF16)})
    return maps_a, maps_b, bias0, np.asarray(b1, np.float32)


def _mk_nc():
    import concourse.bacc as bacc

    # Bacc (not raw Bass): its compile() runs move_matmul_waits_to_ldweights /
    # generate_event_semaphores, required for the 1-wait-per-instruction
    # hardware constraint.
    return bacc.Bacc("TRN2", target_bir_lowering=False, debug=False, num_devices=NCORES)


def _build_phase_a():
    """Convs + sharded shared-matvec partial. Output: part [1, 304]."""
    import concourse.tile as tile
    from concourse import mybir

    f32 = mybir.dt.float32
    bf16 = mybir.dt.bfloat16
    AF = mybir.ActivationFunctionType
    nc = _mk_nc()

    def din(name, shape, dt=f32):
        return nc.dram_tensor(name, list(shape), dt, kind="ExternalInput").ap()

    caw_d = din("caw", (96, CAW_W), bf16)
    cam_d = din("cam", (96, CAM_W), f32)
    w1ps_d = din("w1ps", (128, 32 * 304), bf16)
    part_d = nc.dram_tensor("part", [1, 304], f32, kind="ExternalOutput").ap()

    with tile.TileContext(nc) as tc, ExitStack() as ctx:
        cpool = ctx.enter_context(tc.tile_pool(name="consts", bufs=1))
        spool = ctx.enter_context(tc.tile_pool(name="work", bufs=2))
        pconv = ctx.enter_context(tc.tile_pool(name="pconv", bufs=3, space="PSUM"))
        pmv = ctx.enter_context(tc.tile_pool(name="pmv", bufs=1, space="PSUM"))
        pwarm = ctx.enter_context(tc.tile_pool(name="pwarm", bufs=1, space="PSUM"))

        # warm the ScalarE activation-function table early (overlaps DMAs)
        scr = cpool.tile([1, 1], f32, tag="scr")
        nc.vector.memset(scr[:], 0.0)
        scr2 = cpool.tile([1, 1], f32, tag="scr2")
        nc.scalar.copy(scr2[:], scr[:])
        nc.scalar.activation(scr2[:], scr[:], AF.Relu)

        # PE warm-up: full-array matmuls on a zeroed tile during the DMA head
        # so HAM un-throttles (1.2 -> 2.4 GHz) before the real matmuls arrive.
        # K=1 spins do not register as PE activity; K=128/M=128 do.  Sized to
        # end (~8us) just before the first conv matmul becomes ready (~9.5us).
        wj = cpool.tile([128, 512], bf16, tag="wj")
        nc.vector.memset(wj[:], 0.0)
        wps = pwarm.tile([128, 512], f32, tag="wps")
        for i in range(8):
            nc.tensor.matmul(wps[:], lhsT=wj[:, 0:128], rhs=wj[:], start=True, stop=True)

        # packed consts: ONE DMA each (per-DMA sequencer overhead is ~1.5us)
        cam_t = cpool.tile([96, CAM_W], f32, tag="cam")
        nc.scalar.dma_start(cam_t[:], cam_d)
        caw_t = cpool.tile([96, CAW_W], bf16, tag="caw")
        nc.sync.dma_start(caw_t[:], caw_d)

        # w1 stream: 4 chunk DMAs into one [128, 9728] tile, on the same SP
        # queue AFTER the control DMAs (queue FIFO keeps the small loads first)
        wst = cpool.tile([128, 32 * 304], bf16, tag="w1s")
        CH = 4
        chw = 32 * 304 // CH
        for c in range(CH):
            nc.sync.dma_start(wst[:, c * chw : (c + 1) * chw], w1ps_d[:, c * chw : (c + 1) * chw])

        def s1ap(dx, base):  # stationary for conv1, at partition base 0 or 32
            return caw_t[base : base + 18, 128 * dx : 128 * (dx + 1)]

        def s2ap(dx):
            return caw_t[0:80, 384 + 128 * dx : 384 + 128 * (dx + 1)]

        def s3ap(dx):
            return caw_t[0:96, 768 + 128 * dx : 768 + 128 * (dx + 1)]

        mk1 = lambda b, n=64: cam_t[0:n, 0 + b : 1 + b]
        bm1 = lambda b, n=64: cam_t[0:n, 3 + b : 4 + b]
        mk2 = lambda b, n=64: cam_t[0:n, 6 + b : 7 + b]
        bm2 = lambda b, n=64: cam_t[0:n, 9 + b : 10 + b]
        bc3 = cam_t[0:64, 12:13]

        # next-layer moving-window tiles (built in place by ScalarE writes)
        m2 = [cpool.tile([80, 130], bf16, tag=f"m2_{i}", name=f"m2_{i}") for i in range(3)]
        m3 = [cpool.tile([96, 66], bf16, tag=f"m3_{i}", name=f"m3_{i}") for i in range(2)]
        xc_t = cpool.tile([128, 32], bf16, tag="xc")
        for t in m2:
            nc.vector.memset(t[:], 0.0)
        for t in m3:
            nc.vector.memset(t[:], 0.0)

        def pool_to(ps, width):
            """psum [128, width] (m = (s, pair, c)) -> [64, width//2] max-pooled."""
            vtop = spool.tile([64, width], f32, tag=f"vt{width}")
            nc.scalar.copy(vtop[:], ps[0:64, :])
            v = spool.tile([64, width], f32, tag=f"v{width}")
            nc.vector.tensor_max(v[:], ps[64:128, :], vtop[:])
            vv = v[:].rearrange("p (x t) -> p x t", t=2)
            ph = spool.tile([64, width // 2], f32, tag=f"ph{width}")
            nc.vector.tensor_max(ph[:], vv[:, :, 0], vv[:, :, 1])
            return ph

        # ---- conv1: 3 blocks of 16 output rows -> M2 tiles
        # (row-offset into caw, col-offset of the image window, stationary base)
        win1 = [(0, XS_O, 0), (0, XS2_O, 0), (32, XS_O, 32)]
        for b in range(3):
            rlo, colo, base = win1[b]
            ps = pconv.tile([128, 256], f32, tag="cps")
            for dx in range(3):
                nc.tensor.matmul(
                    ps[:],
                    lhsT=s1ap(dx, base),
                    rhs=caw_t[rlo : rlo + 18, colo + dx : colo + dx + 256],
                    start=(dx == 0),
                    stop=(dx == 2),
                )
            ph = pool_to(ps, 256)  # [64, 128]: partition = jp*8+c, row = 8b+jp
            nc.scalar.activation(
                m2[b][0:64, 1:129], ph[:], AF.Relu, bias=bm1(b), scale=mk1(b)
            )
            if b >= 1:  # rows 8b, 8b+1 also tail rows 8..10 of previous window
                nc.scalar.activation(
                    m2[b - 1][64:80, 1:129],
                    ph[0:16, :],
                    AF.Relu,
                    bias=bm1(b, 16),
                    scale=mk1(b, 16),
                )

        # ---- conv2: 3 blocks of 8 output rows -> M3 tiles
        for b in range(3):
            ps = pconv.tile([128, 128], f32, tag="cps")
            for dx in range(3):
                nc.tensor.matmul(
                    ps[:],
                    lhsT=s2ap(dx),
                    rhs=m2[b][:, dx : dx + 128],
                    start=(dx == 0),
                    stop=(dx == 2),
                )
            ph = pool_to(ps, 128)  # [64, 64]: partition = jp'*16+co, row = 4b+jp'
            if b == 0:
                nc.scalar.activation(m3[0][0:64, 1:65], ph[:], AF.Relu, bias=bm2(0), scale=mk2(0))
            elif b == 1:
                nc.scalar.activation(m3[1][0:64, 1:65], ph[:], AF.Relu, bias=bm2(1), scale=mk2(1))
                nc.scalar.activation(
                    m3[0][64:96, 1:65], ph[0:32, :], AF.Relu, bias=bm2(1, 32), scale=mk2(1, 32)
                )
            else:
                nc.scalar.activation(
                    m3[1][64:96, 1:65], ph[0:32, :], AF.Relu, bias=bm2(2, 32), scale=mk2(2, 32)
                )

        # ---- conv3: 2 m-blocks of 4 output rows -> xc [128, 32]
        for g in range(2):
            ps = pconv.tile([128, 64], f32, tag="cps")
            for dx in range(3):
                nc.tensor.matmul(
                    ps[:],
                    lhsT=s3ap(dx),
                    rhs=m3[g][:, dx : dx + 64],
                    start=(dx == 0),
                    stop=(dx == 2),
                )
            ph = pool_to(ps, 64)  # [64, 32]
            nc.scalar.activation(xc_t[64 * g : 64 * g + 64, :], ph[:], AF.Relu, bias=bc3)

        # ---- shared matvec partial [1, 304]
        ps_mv = pmv.tile([1, 304], f32, tag="mv")
        for b in range(32):
            nc.tensor.matmul(
                ps_mv[:],
                lhsT=xc_t[:, b : b + 1],
                rhs=wst[:, 304 * b : 304 * (b + 1)],
                start=(b == 0),
                stop=(b == 31),
            )
        part_s = spool.tile([1, 304], f32, tag="part")
        nc.scalar.copy(part_s[:], ps_mv[:])
        nc.sync.dma_start(part_d, part_s[:])

    nc.compile()
    return nc


def _build_phase_b():
    """Patch FC for this core's 512 patches, given summed shared vector."""
    import concourse.tile as tile
    from concourse import mybir

    f32 = mybir.dt.float32
    bf16 = mybir.dt.bfloat16
    AF = mybir.ActivationFunctionType
    nc = _mk_nc()

    cbw_d = nc.dram_tensor("cbw", [128, CBW_W], bf16, kind="ExternalInput").ap()
    shcb_d = nc.dram_tensor("shcb", [128, SHCB_W], f32, kind="ExternalInput").ap()
    yout_d = nc.dram_tensor("yout", [48, 512], f32, kind="ExternalOutput").ap()

    mblk = [(0, 128), (128, 128), (256, 48)]
    qblk = [(0, 128), (128, 48)]

    with tile.TileContext(nc) as tc, ExitStack() as ctx:
        cpool = ctx.enter_context(tc.tile_pool(name="consts", bufs=1))
        fpool = ctx.enter_context(tc.tile_pool(name="fc", bufs=1))
        pfc = ctx.enter_context(tc.tile_pool(name="pfc", bufs=1, space="PSUM"))
        phh = ctx.enter_context(tc.tile_pool(name="phh", bufs=3, space="PSUM"))
        pwarm = ctx.enter_context(tc.tile_pool(name="pwarm", bufs=1, space="PSUM"))

        # warm the ScalarE activation-function table early (overlaps DMAs)
        scr = cpool.tile([1, 1], f32, tag="scr")
        nc.vector.memset(scr[:], 0.0)
        scr2 = cpool.tile([1, 1], f32, tag="scr2")
        nc.scalar.activation(scr2[:], scr[:], AF.Relu)
        nc.scalar.activation(scr2[:], scr[:], AF.Sigmoid)

        # PE warm-up during the DMA head (full-array zeros; see phase A)
        wj = cpool.tile([128, 512], bf16, tag="wj")
        nc.vector.memset(wj[:], 0.0)
        wps = pwarm.tile([128, 512], f32, tag="wps")
        for i in range(8):
            nc.tensor.matmul(wps[:], lhsT=wj[:, 0:128], rhs=wj[:], start=True, stop=True)

        cbw = cpool.tile([128, CBW_W], bf16, tag="cbw")
        nc.sync.dma_start(cbw[:], cbw_d)
        shcb = cpool.tile([128, SHCB_W], f32, tag="shcb")
        nc.scalar.dma_start(shcb[:], shcb_d)

        extrasT = cbw[0:18, 0:512]
        w1eT = cbw[0:18, 512:816]
        w2T_t = [cbw[0:128, 816:992], cbw[0:128, 992:1168], cbw[0:48, 1168:1344]]
        w3T_t = [cbw[0:128, 1344:1392], cbw[0:48, 1392:1440]]
        b2c_t = [shcb[0:128, 3:4], shcb[0:48, 4:5]]
        b3c_t = shcb[0:48, 5:6]
        sh_t = [shcb[0:128, 0:1], shcb[0:128, 1:2], shcb[0:48, 2:3]]

        h1_t = []
        for i, (off, mb) in enumerate(mblk):
            ps_e = pfc.tile([mb, 512], f32, tag=f"pse{i}")
            nc.tensor.matmul(
                ps_e[:],
                lhsT=w1eT[:, off : off + mb],
                rhs=extrasT,
                start=True,
                stop=True,
            )
            h1 = fpool.tile([mb, 512], bf16, tag=f"h1{i}")
            from concourse import mybir as _mb
            nc.vector.tensor_scalar(h1[:], ps_e[:], sh_t[i], 0.0, _mb.AluOpType.add, _mb.AluOpType.max)
            h1_t.append(h1)

        h2_t = []
        for q, (qoff, mq) in enumerate(qblk):
            ps_h = phh.tile([mq, 512], f32, tag="psh")
            for i, (off, mb) in enumerate(mblk):
                nc.tensor.matmul(
                    ps_h[:],
                    lhsT=w2T_t[i][:, qoff : qoff + mq],
                    rhs=h1_t[i][:],
                    start=(i == 0),
                    stop=(i == 2),
                )
            h2 = fpool.tile([mq, 512], bf16, tag=f"h2{q}")
            nc.scalar.activation(h2[:], ps_h[:], AF.Relu, bias=b2c_t[q])
            h2_t.append(h2)

        ps_o = phh.tile([48, 512], f32, tag="psh")
        for q, (qoff, mq) in enumerate(qblk):
            nc.tensor.matmul(
                ps_o[:],
                lhsT=w3T_t[q],
                rhs=h2_t[q][:],
                start=(q == 0),
                stop=(q == 1),
            )
        outs = fpool.tile([48, 512], f32, tag="outs")
        nc.scalar.activation(outs[:], ps_o[:], AF.Sigmoid, bias=b3c_t)
        nc.sync.dma_start(yout_d, outs[:])

    nc.compile()
    return nc


def _shcb_pack(sh, bias0):
    shcb = np.zeros((128, SHCB_W), np.float32)
    shcb[0:128, 0] = sh[0:128]
    shcb[0:128, 1] = sh[128:256]
    shcb[0:48, 2] = sh[256:304]
    shcb[:, 3:6] = bias0
    return shcb


def _run(maps_a, maps_b, bias0, b1, trace=False, trace_cores=None):
    from concourse.bass_utils import run_bass_kernel_spmd

    nca = _build_phase_a()
    res_a = run_bass_kernel_spmd(
        nca, maps_a, list(range(NCORES)), trace=trace, trace_cores=trace_cores
    )
    sh = np.sum([res_a.results[r]["part"][0] for r in range(NCORES)], axis=0) + b1
    shcb = _shcb_pack(sh, bias0)
    for mb in maps_b:
        mb["shcb"] = shcb
    ncb = _build_phase_b()
    res_b = run_bass_kernel_spmd(
        ncb, maps_b, list(range(NCORES)), trace=trace, trace_cores=trace_cores
    )
    full = np.empty((G * G, OUT), np.float32)
    for r in range(NCORES):
        full[512 * r : 512 * (r + 1), :] = res_b.results[r]["yout"].T
    return full.reshape(3, IMG, IMG), res_a, res_b


def kernel(**inputs):
    maps_a, maps_b, bias0, b1 = _host_inputs(**inputs)
    out, _, _ = _run(maps_a, maps_b, bias0, b1)
    return out


if __name__ == "__main__":
    import reference

    inp = {k: np.asarray(v) for k, v in reference.setup_inputs().items()}
    got = kernel(**inp)
    exp = np.asarray(reference.reference(**reference.setup_inputs()))
    err = np.abs(got - exp).max() / max(np.abs(exp).max(), 1e-9)
    print("Relative error:", err)


# revision 17
# speedup vs baseline: 1.0624x; 1.0075x over previous
"""ColorizationNet Trainium2 kernel (8 NeuronCores, SPMD, two phases).

Structure exploited: rows of the big FC input [4096, 32786] share an identical
x_conv prefix (32768 cols), so

    fc_in @ w1.T = x_conv @ w1[:, :32768].T  (one shared matvec, [304])
                 + [pos|chunks] @ w1[:, 32768:].T  ([4096,18] GEMM)

Sharding (core r of 8):
  - conv backbone row-sharded: core r produces the x_conv slice for pooled
    rows [4r, 4r+4) of every channel (halos via zero-padded input windows,
    out-of-image "phantom" rows masked to zero via activation scale).
  - shared matvec K-sharded to match (each core streams 1/8 of w1's big
    part, 2.5 MB bf16, laid out so each SBUF partition's data is contiguous
    in DRAM).  Phase A outputs the 8 partials [304]; the host sums them
    (collectives are unavailable under the axon PJRT execution path).
  - phase B: patch FC sharded by patch row, core r handles patches
    [512r, 512(r+1)).

All matmul operands are bf16 (1 cycle/row on PE at any free size, vs 4 for
fp32); accumulation is fp32 in PSUM, and activation bias/scale constants
stay fp32.  Convs use a banded-rows formulation: moving operand = input
rows on SBUF partitions, stationary = banded weight matrix with output
columns m = (s, rowpair, c_out) so the 2x2 maxpool's vertical pair is
partitions p / p+64 (one tensor_max) and the horizontal pair is a stride-2
free-dim pair.  Each layer's pooled activation is written by ScalarE
directly into the next layer's moving-window tiles (no DRAM round trips).
"""

import sys

for _p in ("/opt/trn_rl_repo",):
    if _p not in sys.path:
        sys.path.insert(0, _p)

import numpy as np
import ml_dtypes
from contextlib import ExitStack

BF16 = ml_dtypes.bfloat16

IMG = 256
CS = 4
G = 64
H1 = 304
H2 = 176
OUT = 48
NCORES = 8

# phase-A packed bf16 consts (ONE DMA): [96, 1668]
#   s1 [18, 384] @0 (+ copy at rows 32..50), xs [50, 258] @1152,
#   xs2 [34, 258] @1410, s2 [80, 384] @384, s3 [96, 384] @768
# phase-A packed fp32 masks/biases: [96, 13]
#   mk1 [64,3] @0, bm1 [64,3] @3, mk2 [64,3] @6, bm2 [64,3] @9, bc3 [64,1] @12
CAW_W = 1152 + 258 + 258
XS_O = 1152
XS2_O = 1410
CAM_W = 13

# phase-B packed bf16 consts (ONE DMA): [128, 1440]
#   extrasT [18, 512] @0, w1eT [18, 304] @512, w2a/b [128, 176] @816/@992,
#   w2c [48, 176] @1168, w3a [128, 48] @1344, w3b [48, 48] @1392
# phase-B fp32 vector (ONE DMA, sent after phase A): [128, 6]
#   shc cols 0:3, b2a col 3, b2b col 4, b3 col 5
CBW_W = 1440
SHCB_W = 6


def _build_s1(c1_w):
    # [18, 3, 128]: rows i = in-row in window; cols m = s*64 + jp*8 + c
    s1 = np.zeros((18, 3, 128), np.float32)
    for dx in range(3):
        for s in range(2):
            for jp in range(8):
                j = 2 * jp + s
                for c in range(8):
                    m = s * 64 + jp * 8 + c
                    for dy in range(3):
                        s1[j + dy, dx, m] = c1_w[c, 0, dy, dx]
    return np.ascontiguousarray(s1.reshape(18, 3 * 128))


def _build_s2(c2_w):
    # [80, 3, 128]: rows k = delta*8 + ci (ci in 0..8); cols m = s*64+jp*16+co
    s2 = np.zeros((80, 3, 128), np.float32)
    for dx in range(3):
        for s in range(2):
            for jp in range(4):
                j2 = 2 * jp + s
                for co in range(16):
                    m = s * 64 + jp * 16 + co
                    for ci in range(8):
                        for dy in range(3):
                            s2[(j2 + dy) * 8 + ci, dx, m] = c2_w[co, ci, dy, dx]
    return np.ascontiguousarray(s2.reshape(80, 3 * 128))


def _build_s3(c3_w):
    # [96, 3, 128]: rows k = delta*16 + ci (ci in 0..16); cols m = s*64+jpp*32+co
    s3 = np.zeros((96, 3, 128), np.float32)
    for dx in range(3):
        for s in range(2):
            for jpp in range(2):
                j3 = 2 * jpp + s
                for co in range(32):
                    m = s * 64 + jpp * 32 + co
                    for ci in range(16):
                        for dy in range(3):
                            s3[(j3 + dy) * 16 + ci, dx, m] = c3_w[co, ci, dy, dx]
    return np.ascontiguousarray(s3.reshape(96, 3 * 128))


def _host_inputs(x, c1_w, c1_b, c2_w, c2_b, c3_w, c3_b, w1, b1, w2, b2, w3, b3):
    """Returns (in_maps_a, in_maps_b_partial, b1). Each phase-A map has
    'caw' [96, CAW_W] bf16 (weights + image windows), 'cam' [96, CAM_W] f32,
    'w1ps' [128, 9728] bf16.  Each phase-B map has 'cbw' [128, CBW_W] bf16;
    'shcb' [128, 6] f32 (shared vector + biases) is added after phase A."""
    x = np.asarray(x, np.float32).reshape(IMG, IMG)
    s1 = _build_s1(np.asarray(c1_w, np.float32))
    s2 = _build_s2(np.asarray(c2_w, np.float32))
    s3 = _build_s3(np.asarray(c3_w, np.float32))
    bc3 = np.tile(np.asarray(c3_b, np.float32), 2).reshape(64, 1)

    caw0 = np.zeros((96, CAW_W), np.float32)
    caw0[0:18, 0:384] = s1
    caw0[32:50, 0:384] = s1  # duplicate for the base-32 conv1 window
    caw0[0:80, 384:768] = s2
    caw0[0:96, 768:1152] = s3
    caw0 = caw0.astype(BF16)

    # phase-B packed consts (same for every core except extrasT)
    cbw0 = np.zeros((128, CBW_W), np.float32)
    w1eT = np.asarray(w1, np.float32)[:, 32768:].T  # [18, 304]
    w2T = np.asarray(w2, np.float32).T  # [304, 176]
    w3T = np.asarray(w3, np.float32).T  # [176, 48]
    cbw0[0:18, 512:816] = w1eT
    cbw0[0:128, 816:992] = w2T[0:128]
    cbw0[0:128, 992:1168] = w2T[128:256]
    cbw0[0:48, 1168:1344] = w2T[256:304]
    cbw0[0:128, 1344:1392] = w3T[0:128]
    cbw0[0:48, 1392:1440] = w3T[128:176]
    bias0 = np.zeros((128, 3), np.float32)
    bias0[0:128, 0] = np.asarray(b2, np.float32)[0:128]
    bias0[0:48, 1] = np.asarray(b2, np.float32)[128:176]
    bias0[0:48, 2] = np.asarray(b3, np.float32)

    w1bigT = np.ascontiguousarray(np.asarray(w1, np.float32)[:, :32768].T)  # [32768, 304]
    chunks = x.reshape(G, CS, G, CS).transpose(0, 2, 1, 3).reshape(G * G, CS * CS)
    pi = (np.arange(G * G) // G).astype(np.float32) * CS
    pj = (np.arange(G * G) % G).astype(np.float32) * CS

    P = np.arange(128)
    B = np.arange(32)
    c1b = np.asarray(c1_b, np.float32)
    c2b = np.asarray(c2_b, np.float32)

    maps_a, maps_b = [], []
    for r in range(NCORES):
        # xs: x rows [32r-7, 32r+43), cols padded by 1 each side
        xs = np.zeros((50, 258), np.float32)
        lo = 32 * r - 7
        hi = 32 * r + 43
        slo, shi = max(lo, 0), min(hi, IMG)
        xs[slo - lo : shi - lo, 1:257] = x[slo:shi, :]
        xs = xs.astype(BF16)
        caw = caw0.copy()
        caw[0:50, XS_O : XS_O + 258] = xs
        caw[0:34, XS2_O : XS2_O + 258] = xs[16:50]

        cam = np.zeros((96, CAM_W), np.float32)
        # row-validity masks (zero out-of-image "phantom" pooled rows)
        for b in range(3):
            for jp in range(8):
                valid = 0 <= (16 * r - 3 + 8 * b + jp) < 128
                cam[jp * 8 : jp * 8 + 8, 0 + b] = 1.0 if valid else 0.0
                cam[jp * 8 : jp * 8 + 8, 3 + b] = c1b if valid else 0.0
            for jp in range(4):
                valid = 0 <= (8 * r - 1 + 4 * b + jp) < 64
                cam[jp * 16 : jp * 16 + 16, 6 + b] = 1.0 if valid else 0.0
                cam[jp * 16 : jp * 16 + 16, 9 + b] = c2b if valid else 0.0
        cam[0:64, 12:13] = bc3

        # w1ps [128, 32*304]: w1ps[p, j*304+o] = w1[o, kglobal(p, j)],
        # kglobal = (p%32)*1024 + (4r + p//32)*32 + j
        kg = (P[None, :] % 32) * 1024 + (4 * r + P[None, :] // 32) * 32 + B[:, None]
        w1ps = np.ascontiguousarray(
            w1bigT[kg.ravel()].reshape(32, 128, 304).transpose(1, 0, 2).reshape(128, 32 * 304)
        ).astype(BF16)
        maps_a.append({"caw": caw, "cam": cam, "w1ps": w1ps})

        cbw = cbw0.copy()
        sl = slice(512 * r, 512 * (r + 1))
        cbw[0, 0:512] = pi[sl]
        cbw[1, 0:512] = pj[sl]
        cbw[2:18, 0:512] = chunks[sl].T
        maps_b.append({"cbw": cbw.astype(BF16)})
    return maps_a, maps_b, bias0, np.asarray(b1, np.float32)


def _mk_nc():
    import concourse.bacc as bacc

    # Bacc (not raw Bass): its compile() runs move_matmul_waits_to_ldweights /
    # generate_event_semaphores, required for the 1-wait-per-instruction
    # hardware constraint.
    return bacc.Bacc("TRN2", target_bir_lowering=False, debug=False, num_devices=NCORES)


def _build_phase_a():
    """Convs + sharded shared-matvec partial. Output: part [1, 304]."""
    import concourse.tile as tile
    from concourse import mybir

    f32 = mybir.dt.float32
    bf16 = mybir.dt.bfloat16
    AF = mybir.ActivationFunctionType
    nc = _mk_nc()

    def din(name, shape, dt=f32):
        return nc.dram_tensor(name, list(shape), dt, kind="ExternalInput").ap()

    caw_d = din("caw", (96, CAW_W), bf16)
    cam_d = din("cam", (96, CAM_W), f32)
    w1ps_d = din("w1ps", (128, 32 * 304), bf16)
    part_d = nc.dram_tensor("part", [1, 304], f32, kind="ExternalOutput").ap()

    with tile.TileContext(nc) as tc, ExitStack() as ctx:
        cpool = ctx.enter_context(tc.tile_pool(name="consts", bufs=1))
        spool = ctx.enter_context(tc.tile_pool(name="work", bufs=2))
        pconv = ctx.enter_context(tc.tile_pool(name="pconv", bufs=3, space="PSUM"))
        pmv = ctx.enter_context(tc.tile_pool(name="pmv", bufs=1, space="PSUM"))

        # warm the ScalarE activation-function table early (overlaps DMAs)
        scr = cpool.tile([1, 1], f32, tag="scr")
        nc.vector.memset(scr[:], 0.0)
        scr2 = cpool.tile([1, 1], f32, tag="scr2")
        nc.scalar.copy(scr2[:], scr[:])
        nc.scalar.activation(scr2[:], scr[:], AF.Relu)

        # packed consts: ONE DMA each (per-DMA sequencer overhead is ~1.5us)
        cam_t = cpool.tile([96, CAM_W], f32, tag="cam")
        nc.scalar.dma_start(cam_t[:], cam_d)
        caw_t = cpool.tile([96, CAW_W], bf16, tag="caw")
        nc.sync.dma_start(caw_t[:], caw_d)

        # w1 stream: 4 chunk DMAs into one [128, 9728] tile, on the same SP
        # queue AFTER the control DMAs (queue FIFO keeps the small loads first)
        wst = cpool.tile([128, 32 * 304], bf16, tag="w1s")
        CH = 4
        chw = 32 * 304 // CH
        for c in range(CH):
            nc.sync.dma_start(wst[:, c * chw : (c + 1) * chw], w1ps_d[:, c * chw : (c + 1) * chw])

        def s1ap(dx, base):  # stationary for conv1, at partition base 0 or 32
            return caw_t[base : base + 18, 128 * dx : 128 * (dx + 1)]

        def s2ap(dx):
            return caw_t[0:80, 384 + 128 * dx : 384 + 128 * (dx + 1)]

        def s3ap(dx):
            return caw_t[0:96, 768 + 128 * dx : 768 + 128 * (dx + 1)]

        mk1 = lambda b, n=64: cam_t[0:n, 0 + b : 1 + b]
        bm1 = lambda b, n=64: cam_t[0:n, 3 + b : 4 + b]
        mk2 = lambda b, n=64: cam_t[0:n, 6 + b : 7 + b]
        bm2 = lambda b, n=64: cam_t[0:n, 9 + b : 10 + b]
        bc3 = cam_t[0:64, 12:13]

        # next-layer moving-window tiles (built in place by ScalarE writes)
        m2 = [cpool.tile([80, 130], bf16, tag=f"m2_{i}", name=f"m2_{i}") for i in range(3)]
        m3 = [cpool.tile([96, 66], bf16, tag=f"m3_{i}", name=f"m3_{i}") for i in range(2)]
        xc_t = cpool.tile([128, 32], bf16, tag="xc")
        for t in m2:
            nc.vector.memset(t[:], 0.0)
        for t in m3:
            nc.vector.memset(t[:], 0.0)

        def pool_to(ps, width):
            """psum [128, width] (m = (s, pair, c)) -> [64, width//2] max-pooled."""
            vtop = spool.tile([64, width], f32, tag=f"vt{width}")
            nc.scalar.copy(vtop[:], ps[0:64, :])
            v = spool.tile([64, width], f32, tag=f"v{width}")
            nc.vector.tensor_max(v[:], ps[64:128, :], vtop[:])
            vv = v[:].rearrange("p (x t) -> p x t", t=2)
            ph = spool.tile([64, width // 2], f32, tag=f"ph{width}")
            nc.vector.tensor_max(ph[:], vv[:, :, 0], vv[:, :, 1])
            return ph

        # ---- conv1: 3 blocks of 16 output rows -> M2 tiles
        # (row-offset into caw, col-offset of the image window, stationary base)
        win1 = [(0, XS_O, 0), (0, XS2_O, 0), (32, XS_O, 32)]
        for b in range(3):
            rlo, colo, base = win1[b]
            ps = pconv.tile([128, 256], f32, tag="cps")
            for dx in range(3):
                nc.tensor.matmul(
                    ps[:],
                    lhsT=s1ap(dx, base),
                    rhs=caw_t[rlo : rlo + 18, colo + dx : colo + dx + 256],
                    start=(dx == 0),
                    stop=(dx == 2),
                )
            ph = pool_to(ps, 256)  # [64, 128]: partition = jp*8+c, row = 8b+jp
            nc.scalar.activation(
                m2[b][0:64, 1:129], ph[:], AF.Relu, bias=bm1(b), scale=mk1(b)
            )
            if b >= 1:  # rows 8b, 8b+1 also tail rows 8..10 of previous window
                nc.scalar.activation(
                    m2[b - 1][64:80, 1:129],
                    ph[0:16, :],
                    AF.Relu,
                    bias=bm1(b, 16),
                    scale=mk1(b, 16),
                )

        # ---- conv2: 3 blocks of 8 output rows -> M3 tiles
        for b in range(3):
            ps = pconv.tile([128, 128], f32, tag="cps")
            for dx in range(3):
                nc.tensor.matmul(
                    ps[:],
                    lhsT=s2ap(dx),
                    rhs=m2[b][:, dx : dx + 128],
                    start=(dx == 0),
                    stop=(dx == 2),
                )
            ph = pool_to(ps, 128)  # [64, 64]: partition = jp'*16+co, row = 4b+jp'
            if b == 0:
                nc.scalar.activation(m3[0][0:64, 1:65], ph[:], AF.Relu, bias=bm2(0), scale=mk2(0))
            elif b == 1:
                nc.scalar.activation(m3[1][0:64, 1:65], ph[:], AF.Relu, bias=bm2(1), scale=mk2(1))
                nc.scalar.activation(
                    m3[0][64:96, 1:65], ph[0:32, :], AF.Relu, bias=bm2(1, 32), scale=mk2(1, 32)
                )
            else:
                nc.scalar.activation(
                    m3[1][64:96, 1:65], ph[0:32, :], AF.Relu, bias=bm2(2, 32), scale=mk2(2, 32)
                )

        # ---- conv3: 2 m-blocks of 4 output rows -> xc [128, 32]
        for g in range(2):
            ps = pconv.tile([128, 64], f32, tag="cps")
            for dx in range(3):
                nc.tensor.matmul(
                    ps[:],
                    lhsT=s3ap(dx),
                    rhs=m3[g][:, dx : dx + 64],
                    start=(dx == 0),
                    stop=(dx == 2),
                )
            ph = pool_to(ps, 64)  # [64, 32]
            nc.scalar.activation(xc_t[64 * g : 64 * g + 64, :], ph[:], AF.Relu, bias=bc3)

        # ---- shared matvec partial [1, 304]
        ps_mv = pmv.tile([1, 304], f32, tag="mv")
        for b in range(32):
            nc.tensor.matmul(
                ps_mv[:],
                lhsT=xc_t[:, b : b + 1],
                rhs=wst[:, 304 * b : 304 * (b + 1)],
                start=(b == 0),
                stop=(b == 31),
            )
        part_s = spool.tile([1, 304], f32, tag="part")
        nc.scalar.copy(part_s[:], ps_mv[:])
        nc.sync.dma_start(part_d, part_s[:])

    nc.compile()
    return nc


def _build_phase_b():
    """Patch FC for this core's 512 patches, given summed shared vector."""
    import concourse.tile as tile
    from concourse import mybir

    f32 = mybir.dt.float32
    bf16 = mybir.dt.bfloat16
    AF = mybir.ActivationFunctionType
    nc = _mk_nc()

    cbw_d = nc.dram_tensor("cbw", [128, CBW_W], bf16, kind="ExternalInput").ap()
    shcb_d = nc.dram_tensor("shcb", [128, SHCB_W], f32, kind="ExternalInput").ap()
    yout_d = nc.dram_tensor("yout", [48, 512], f32, kind="ExternalOutput").ap()

    mblk = [(0, 128), (128, 128), (256, 48)]
    qblk = [(0, 128), (128, 48)]

    with tile.TileContext(nc) as tc, ExitStack() as ctx:
        cpool = ctx.enter_context(tc.tile_pool(name="consts", bufs=1))
        fpool = ctx.enter_context(tc.tile_pool(name="fc", bufs=1))
        pfc = ctx.enter_context(tc.tile_pool(name="pfc", bufs=1, space="PSUM"))
        phh = ctx.enter_context(tc.tile_pool(name="phh", bufs=3, space="PSUM"))

        # warm the ScalarE activation-function table early (overlaps DMAs)
        scr = cpool.tile([1, 1], f32, tag="scr")
        nc.vector.memset(scr[:], 0.0)
        scr2 = cpool.tile([1, 1], f32, tag="scr2")
        nc.scalar.activation(scr2[:], scr[:], AF.Relu)
        nc.scalar.activation(scr2[:], scr[:], AF.Sigmoid)

        cbw = cpool.tile([128, CBW_W], bf16, tag="cbw")
        nc.sync.dma_start(cbw[:], cbw_d)
        shcb = cpool.tile([128, SHCB_W], f32, tag="shcb")
        nc.scalar.dma_start(shcb[:], shcb_d)

        extrasT = cbw[0:18, 0:512]
        w1eT = cbw[0:18, 512:816]
        w2T_t = [cbw[0:128, 816:992], cbw[0:128, 992:1168], cbw[0:48, 1168:1344]]
        w3T_t = [cbw[0:128, 1344:1392], cbw[0:48, 1392:1440]]
        b2c_t = [shcb[0:128, 3:4], shcb[0:48, 4:5]]
        b3c_t = shcb[0:48, 5:6]
        sh_t = [shcb[0:128, 0:1], shcb[0:128, 1:2], shcb[0:48, 2:3]]

        h1_t = []
        for i, (off, mb) in enumerate(mblk):
            ps_e = pfc.tile([mb, 512], f32, tag=f"pse{i}")
            nc.tensor.matmul(
                ps_e[:],
                lhsT=w1eT[:, off : off + mb],
                rhs=extrasT,
                start=True,
                stop=True,
            )
            h1 = fpool.tile([mb, 512], bf16, tag=f"h1{i}")
            from concourse import mybir as _mb
            nc.vector.tensor_scalar(h1[:], ps_e[:], sh_t[i], 0.0, _mb.AluOpType.add, _mb.AluOpType.max)
            h1_t.append(h1)

        h2_t = []
        for q, (qoff, mq) in enumerate(qblk):
            ps_h = phh.tile([mq, 512], f32, tag="psh")
            for i, (off, mb) in enumerate(mblk):
                nc.tensor.matmul(
                    ps_h[:],
                    lhsT=w2T_t[i][:, qoff : qoff + mq],
                    rhs=h1_t[i][:],
                    start=(i == 0),
                    stop=(i == 2),
                )
            h2 = fpool.tile([mq, 512], bf16, tag=f"h2{q}")
            nc.scalar.activation(h2[:], ps_h[:], AF.Relu, bias=b2c_t[q])
            h2_t.append(h2)

        ps_o = phh.tile([48, 512], f32, tag="psh")
        for q, (qoff, mq) in enumerate(qblk):
            nc.tensor.matmul(
                ps_o[:],
                lhsT=w3T_t[q],
                rhs=h2_t[q][:],
                start=(q == 0),
                stop=(q == 1),
            )
        outs = fpool.tile([48, 512], f32, tag="outs")
        nc.scalar.activation(outs[:], ps_o[:], AF.Sigmoid, bias=b3c_t)
        nc.sync.dma_start(yout_d, outs[:])

    nc.compile()
    return nc


def _shcb_pack(sh, bias0):
    shcb = np.zeros((128, SHCB_W), np.float32)
    shcb[0:128, 0] = sh[0:128]
    shcb[0:128, 1] = sh[128:256]
    shcb[0:48, 2] = sh[256:304]
    shcb[:, 3:6] = bias0
    return shcb


def _run(maps_a, maps_b, bias0, b1, trace=False, trace_cores=None):
    from concourse.bass_utils import run_bass_kernel_spmd

    nca = _build_phase_a()
    res_a = run_bass_kernel_spmd(
        nca, maps_a, list(range(NCORES)), trace=trace, trace_cores=trace_cores
    )
    sh = np.sum([res_a.results[r]["part"][0] for r in range(NCORES)], axis=0) + b1
    shcb = _shcb_pack(sh, bias0)
    for mb in maps_b:
        mb["shcb"] = shcb
    ncb = _build_phase_b()
    res_b = run_bass_kernel_spmd(
        ncb, maps_b, list(range(NCORES)), trace=trace, trace_cores=trace_cores
    )
    full = np.empty((G * G, OUT), np.float32)
    for r in range(NCORES):
        full[512 * r : 512 * (r + 1), :] = res_b.results[r]["yout"].T
    return full.reshape(3, IMG, IMG), res_a, res_b


def kernel(**inputs):
    maps_a, maps_b, bias0, b1 = _host_inputs(**inputs)
    out, _, _ = _run(maps_a, maps_b, bias0, b1)
    return out


if __name__ == "__main__":
    import reference

    inp = {k: np.asarray(v) for k, v in reference.setup_inputs().items()}
    got = kernel(**inp)
    exp = np.asarray(reference.reference(**reference.setup_inputs()))
    err = np.abs(got - exp).max() / max(np.abs(exp).max(), 1e-9)
    print("Relative error:", err)
